# revision 23
# baseline (speedup 1.0000x reference)
"""ContextualConv2d Trainium2 kernel.

out = conv2d(x, weight, pad=1) + (c @ c_weight.T)[:, :, None, None] + bias[None, :, None, None]

Full shapes: x (32,128,64,64) f32, c (32,64), weight (256,128,3,3),
c_weight (256,64), bias (256,) -> out (32,256,64,64).

Strategy: data-parallel over batch across 8 NeuronCores (4 images each).
Per core the conv is an implicit GEMM: each image lives in SBUF as
[128ci, 66*64] (two zero rows for the H halo, no W padding so every DMA
is contiguous). For each 128-wide C_out tile and each 512-column output
block (8 image rows x 64 cols) we accumulate 9 matmuls (one per filter
tap) into a PSUM bank using float32r operands (full PE rate at N>=256).
The +-1 column taps use column-sliced matmuls (N=504) with strided PSUM
sub-APs -- the zero-padding contribution is simply never computed; the
full-width center tap runs first in each accumulation group so every
PSUM element is written. The context bias (c @ c_weight.T + bias) comes
from one small on-device matmul per C_out tile (a ones-row on the rhs
folds in the channel bias) and is fused into the PSUM->SBUF epilogue,
which stages a full [128, 4096] image plane for one contiguous 2MB
output DMA per (image, C_out tile).
"""

import numpy as np

import concourse.tile as tile
from concourse import bacc, bass_utils, mybir

N_CORES = 8
N_FULL = 32
IMG = N_FULL // N_CORES  # images per core
CIN = 128
COUT = 256
H = W = 64
HW = H * W
KDIM = 3
CDIM = 64
XROWS = H + 2  # 2 zero rows for the H halo
CO_TILES = COUT // 128
ROWS_PER_BLK = 8
NBLK = H // ROWS_PER_BLK
BLK_N = ROWS_PER_BLK * W  # 512 = one fp32 PSUM bank
F32 = mybir.dt.float32
F32R = mybir.dt.float32r

_cached_nc = None


def _build():
    nc = bacc.Bacc(
        "TRN2",
        target_bir_lowering=False,
        debug=False,
        enable_asserts=False,
        num_devices=N_CORES,
    )
    x_d = nc.dram_tensor("x", (IMG, CIN, H, W + 1), F32R, kind="ExternalInput").ap()
    wt_d = nc.dram_tensor("wt", (KDIM * KDIM, CIN, COUT), F32R, kind="ExternalInput").ap()
    cb_d = nc.dram_tensor("cb", (CDIM + 1, IMG), F32R, kind="ExternalInput").ap()
    cwb_d = nc.dram_tensor("cwb", (CDIM + 1, COUT), F32R, kind="ExternalInput").ap()
    z_d = nc.dram_tensor("z", (CIN, W + 2), F32R, kind="ExternalInput").ap()
    out_d = nc.dram_tensor("out", (IMG, COUT, H, W), F32, kind="ExternalOutput").ap()

    with tile.TileContext(nc) as tc:
        with (
            tc.tile_pool(name="consts", bufs=1) as consts,
            tc.tile_pool(name="xbuf", bufs=1) as xbuf,
            tc.tile_pool(name="obuf", bufs=2) as obuf,
            tc.tile_pool(name="ps", bufs=5, space="PSUM") as pspool,
            tc.tile_pool(name="cps", bufs=1, space="PSUM") as cpspool,
            tc.tile_pool(name="wps", bufs=1, space="PSUM") as wpspool,
        ):
            # PE warmup: the PE idles for the first ~12us waiting on input
            # DMAs, and the HAM clock gate needs ~3.4us of sustained matmul
            # activity to lift the 1.2GHz cold throttle. Run dummy matmuls
            # on a small scratch tile so the real matmuls start at 2.4GHz.
            # The warmup matmuls only exist to keep the PE busy (HAM
            # un-throttle) while the real input DMAs land; their PSUM bank is
            # never read. bf16 zeros: memset is legal for bf16 and the PE
            # rate is the same.
            warm_sb = consts.tile([CIN, BLK_N], mybir.dt.bfloat16)
            nc.gpsimd.memset(warm_sb[:], 0.0)
            wps = wpspool.tile([128, BLK_N], F32)
            for _ in range(18):
                nc.tensor.matmul(
                    wps[:],
                    lhsT=warm_sb[:, 0:128],
                    rhs=warm_sb[:],
                    start=True,
                    stop=True,
                )

            cwb_sb = consts.tile([CDIM + 1, COUT], F32R)
            nc.sync.dma_start(out=cwb_sb[:], in_=cwb_d)
            cb_sb = consts.tile([CDIM + 1, IMG], F32R)
            nc.sync.dma_start(out=cb_sb[:], in_=cb_d)
            w_sb = consts.tile([CIN, KDIM * KDIM * COUT], F32R)
            nc.sync.dma_start(
                out=w_sb[:].rearrange("p (k o) -> p k o", o=COUT),
                in_=wt_d.transpose([1, 0, 2]),
            )

            # ctxb[t][co, n] = sum_d c_weight[co, d] * c[n, d] + bias[co]
            ctxb = []
            for t in range(CO_TILES):
                cps = cpspool.tile([128, IMG], F32, tag=f"cps{t}")
                nc.tensor.matmul(
                    cps[:],
                    lhsT=cwb_sb[:, t * 128 : (t + 1) * 128],
                    rhs=cb_sb[:],
                    start=True,
                    stop=True,
                )
                csb = consts.tile([128, IMG], F32, tag=f"ctxb{t}")
                nc.vector.tensor_copy(csb[:], cps[:])
                ctxb.append(csb)

            # per-image input planes with stride-65 rows: position
            # 1 + u*PWS + c holds image pixel (u-1, c); column PWS-1 of each
            # row is a zero guard (baked into the host-padded x tensor), and
            # rows 0 / XROWS-1 plus the leading element are zeroed from z_d.
            # The +-1-column taps then read straight through the guards
            # (which contribute zero), so every tap is a uniform N=512
            # matmul with inner-contiguous rhs and a plain 2D PSUM out.
            PWS = W + 1

            def load_image(n):
                """Emit the image-n load: top zero row + leading guard, two
                interior halves, bottom zero row. Fully contiguous DMAs."""
                # one extra row of slack: tap AP slices extend past the last
                # guard before the [:, :, :W] crop trims them
                xp = xbuf.tile([CIN, 1 + (XROWS + 1) * PWS], F32R, tag=f"ximg{n}")
                nc.sync.dma_start(out=xp[:, 0 : 1 + PWS], in_=z_d[:, 0 : 1 + PWS])
                nc.sync.dma_start(
                    out=xp[:, 1 + PWS : 1 + (XROWS - 1) * PWS],
                    in_=x_d[n].rearrange("p h w -> p (h w)"),
                )
                nc.sync.dma_start(
                    out=xp[:, 1 + (XROWS - 1) * PWS : 1 + XROWS * PWS],
                    in_=z_d[:, 0:PWS],
                )
                return xp

            xflats = {0: load_image(0)}

            for n in range(IMG):
                xf = xflats[n]
                for t in range(CO_TILES):
                    obig = obuf.tile([128, HW], F32)
                    for b in range(NBLK):
                        ps = pspool.tile([128, BLK_N], F32)
                        r0 = b * ROWS_PER_BLK
                        for i in range(KDIM * KDIM):
                            kh, kw = divmod(i, KDIM)
                            w0 = i * COUT + t * 128
                            o = 1 + (r0 + kh) * PWS + (kw - 1)
                            rhs = xf[:, o : o + ROWS_PER_BLK * PWS].rearrange(
                                "p (r c) -> p r c", c=PWS
                            )[:, :, :W]
                            nc.tensor.matmul(
                                ps[:],
                                lhsT=w_sb[:, w0 : w0 + 128],
                                rhs=rhs,
                                start=(i == 0),
                                stop=(i == KDIM * KDIM - 1),
                            )
                        oslice = obig[:, b * BLK_N : (b + 1) * BLK_N]
                        if t == 0:
                            nc.scalar.activation(
                                oslice,
                                ps[:],
                                mybir.ActivationFunctionType.Identity,
                                bias=ctxb[t][:, n : n + 1],
                                scale=1.0,
                            )
                        else:
                            nc.vector.tensor_scalar_add(
                                oslice, ps[:], ctxb[t][:, n : n + 1]
                            )
                    # split the 2MB plane store so the last piece doesn't sit
                    # whole on the kernel's critical tail
                    oflat = out_d[n, t * 128 : (t + 1) * 128].rearrange(
                        "o h w -> o (h w)"
                    )
                    for q in range(4):
                        nc.sync.dma_start(
                            out=oflat[:, q * (HW // 4) : (q + 1) * (HW // 4)],
                            in_=obig[:, q * (HW // 4) : (q + 1) * (HW // 4)],
                        )
                    # prefetch the next image while this one's second
                    # C_out tile computes
                    if t == 0 and n + 1 < IMG:
                        xflats[n + 1] = load_image(n + 1)
    nc.compile()
    return nc


def get_nc():
    global _cached_nc
    if _cached_nc is None:
        _cached_nc = _build()
    return _cached_nc


def prep_in_maps(x, c, weight, c_weight, bias):
    x = np.ascontiguousarray(np.asarray(x, dtype=np.float32))
    c = np.asarray(c, dtype=np.float32)
    weight = np.asarray(weight, dtype=np.float32)
    c_weight = np.asarray(c_weight, dtype=np.float32)
    bias = np.asarray(bias, dtype=np.float32)

    wt = np.ascontiguousarray(
        weight.transpose(2, 3, 1, 0).reshape(KDIM * KDIM, CIN, COUT)
    )
    cwb = np.ascontiguousarray(np.concatenate([c_weight.T, bias[None, :]], axis=0))
    z = np.zeros((CIN, W + 2), np.float32)
    xpad = np.zeros((N_FULL, CIN, H, W + 1), np.float32)
    xpad[:, :, :, :W] = x
    in_maps = []
    for i in range(N_CORES):
        xs = np.ascontiguousarray(xpad[i * IMG : (i + 1) * IMG])
        cb = np.ascontiguousarray(
            np.concatenate(
                [c[i * IMG : (i + 1) * IMG].T, np.ones((1, IMG), np.float32)], axis=0
            )
        )
        in_maps.append({"x": xs, "wt": wt, "cb": cb, "cwb": cwb, "z": z})
    return in_maps


def run(x, c, weight, c_weight, bias, trace=False):
    nc = get_nc()
    in_maps = prep_in_maps(x, c, weight, c_weight, bias)
    res = bass_utils.run_bass_kernel_spmd(
        nc, in_maps, core_ids=list(range(N_CORES)), trace=trace
    )
    out = np.concatenate([res.results[i]["out"] for i in range(N_CORES)], axis=0)
    return out, res


def kernel(x, c, weight, c_weight, bias):
    out, _ = run(x, c, weight, c_weight, bias)
    return out


# revision 25
# speedup vs baseline: 1.0121x; 1.0121x over previous
"""ContextualConv2d Trainium2 kernel.

out = conv2d(x, weight, pad=1) + (c @ c_weight.T)[:, :, None, None] + bias[None, :, None, None]

Full shapes: x (32,128,64,64) f32, c (32,64), weight (256,128,3,3),
c_weight (256,64), bias (256,) -> out (32,256,64,64).

Strategy: data-parallel over batch across 8 NeuronCores (4 images each).
Per core the conv is an implicit GEMM: each image lives in SBUF as
[128ci, 66*64] (two zero rows for the H halo, no W padding so every DMA
is contiguous). For each 128-wide C_out tile and each 512-column output
block (8 image rows x 64 cols) we accumulate 9 matmuls (one per filter
tap) into a PSUM bank using float32r operands (full PE rate at N>=256).
The +-1 column taps use column-sliced matmuls (N=504) with strided PSUM
sub-APs -- the zero-padding contribution is simply never computed; the
full-width center tap runs first in each accumulation group so every
PSUM element is written. The context bias (c @ c_weight.T + bias) comes
from one small on-device matmul per C_out tile (a ones-row on the rhs
folds in the channel bias) and is fused into the PSUM->SBUF epilogue,
which stages a full [128, 4096] image plane for one contiguous 2MB
output DMA per (image, C_out tile).
"""

import numpy as np

import concourse.tile as tile
from concourse import bacc, bass_utils, mybir

N_CORES = 8
N_FULL = 32
IMG = N_FULL // N_CORES  # images per core
CIN = 128
COUT = 256
H = W = 64
HW = H * W
KDIM = 3
CDIM = 64
XROWS = H + 2  # 2 zero rows for the H halo
CO_TILES = COUT // 128
ROWS_PER_BLK = 8
NBLK = H // ROWS_PER_BLK
BLK_N = ROWS_PER_BLK * W  # 512 = one fp32 PSUM bank
F32 = mybir.dt.float32
F32R = mybir.dt.float32r

_cached_nc = None


def _build():
    nc = bacc.Bacc(
        "TRN2",
        target_bir_lowering=False,
        debug=False,
        enable_asserts=False,
        num_devices=N_CORES,
    )
    x_d = nc.dram_tensor("x", (IMG, CIN, H, W + 1), F32R, kind="ExternalInput").ap()
    wt_d = nc.dram_tensor("wt", (KDIM * KDIM, CIN, COUT), F32R, kind="ExternalInput").ap()
    cb_d = nc.dram_tensor("cb", (CDIM + 1, IMG), F32R, kind="ExternalInput").ap()
    cwb_d = nc.dram_tensor("cwb", (CDIM + 1, COUT), F32R, kind="ExternalInput").ap()
    z_d = nc.dram_tensor("z", (CIN, W + 2), F32R, kind="ExternalInput").ap()
    out_d = nc.dram_tensor("out", (IMG, COUT, H, W), F32, kind="ExternalOutput").ap()

    with tile.TileContext(nc) as tc:
        with (
            tc.tile_pool(name="consts", bufs=1) as consts,
            tc.tile_pool(name="xbuf", bufs=1) as xbuf,
            tc.tile_pool(name="obuf", bufs=2) as obuf,
            tc.tile_pool(name="ps", bufs=5, space="PSUM") as pspool,
            tc.tile_pool(name="cps", bufs=1, space="PSUM") as cpspool,
            tc.tile_pool(name="wps", bufs=1, space="PSUM") as wpspool,
        ):
            # PE warmup: the PE idles for the first ~12us waiting on input
            # DMAs, and the HAM clock gate needs ~3.4us of sustained matmul
            # activity to lift the 1.2GHz cold throttle. Run dummy matmuls
            # on a small scratch tile so the real matmuls start at 2.4GHz.
            # The warmup matmuls only exist to keep the PE busy (HAM
            # un-throttle) while the real input DMAs land; their PSUM bank is
            # never read. bf16 zeros: memset is legal for bf16 and the PE
            # rate is the same.
            warm_sb = consts.tile([CIN, BLK_N], mybir.dt.bfloat16)
            nc.gpsimd.memset(warm_sb[:], 0.0)
            wps = wpspool.tile([128, BLK_N], F32)
            for _ in range(10):
                nc.tensor.matmul(
                    wps[:],
                    lhsT=warm_sb[:, 0:128],
                    rhs=warm_sb[:],
                    start=True,
                    stop=True,
                )

            cwb_sb = consts.tile([CDIM + 1, COUT], F32R)
            nc.sync.dma_start(out=cwb_sb[:], in_=cwb_d)
            cb_sb = consts.tile([CDIM + 1, IMG], F32R)
            nc.sync.dma_start(out=cb_sb[:], in_=cb_d)
            w_sb = consts.tile([CIN, KDIM * KDIM * COUT], F32R)
            nc.sync.dma_start(
                out=w_sb[:].rearrange("p (k o) -> p k o", o=COUT),
                in_=wt_d.transpose([1, 0, 2]),
            )

            # ctxb[t][co, n] = sum_d c_weight[co, d] * c[n, d] + bias[co]
            ctxb = []
            for t in range(CO_TILES):
                cps = cpspool.tile([128, IMG], F32, tag=f"cps{t}")
                nc.tensor.matmul(
                    cps[:],
                    lhsT=cwb_sb[:, t * 128 : (t + 1) * 128],
                    rhs=cb_sb[:],
                    start=True,
                    stop=True,
                )
                csb = consts.tile([128, IMG], F32, tag=f"ctxb{t}")
                nc.vector.tensor_copy(csb[:], cps[:])
                ctxb.append(csb)

            # per-image input planes with stride-65 rows: position
            # 1 + u*PWS + c holds image pixel (u-1, c); column PWS-1 of each
            # row is a zero guard (baked into the host-padded x tensor), and
            # rows 0 / XROWS-1 plus the leading element are zeroed from z_d.
            # The +-1-column taps then read straight through the guards
            # (which contribute zero), so every tap is a uniform N=512
            # matmul with inner-contiguous rhs and a plain 2D PSUM out.
            PWS = W + 1

            def load_image(n):
                """Emit the image-n load: top zero row + leading guard, two
                interior halves, bottom zero row. Fully contiguous DMAs."""
                # one extra row of slack: tap AP slices extend past the last
                # guard before the [:, :, :W] crop trims them
                # image loads ride the scalar (ACT) HWDGE ring, in parallel
                # with the weight load on the sync ring; the first 16 rows
                # land first so block 0 can start early
                xp = xbuf.tile([CIN, 1 + (XROWS + 1) * PWS], F32R, tag=f"ximg{n}")
                xflat = x_d[n].rearrange("p h w -> p (h w)")
                cut = 16 * PWS
                nc.scalar.dma_start(out=xp[:, 0 : 1 + PWS], in_=z_d[:, 0 : 1 + PWS])
                nc.scalar.dma_start(
                    out=xp[:, 1 + PWS : 1 + PWS + cut], in_=xflat[:, 0:cut]
                )
                nc.scalar.dma_start(
                    out=xp[:, 1 + PWS + cut : 1 + (XROWS - 1) * PWS],
                    in_=xflat[:, cut:],
                )
                nc.scalar.dma_start(
                    out=xp[:, 1 + (XROWS - 1) * PWS : 1 + XROWS * PWS],
                    in_=z_d[:, 0:PWS],
                )
                return xp

            xflats = {0: load_image(0)}

            for n in range(IMG):
                xf = xflats[n]
                for t in range(CO_TILES):
                    obig = obuf.tile([128, HW], F32)
                    for b in range(NBLK):
                        ps = pspool.tile([128, BLK_N], F32)
                        r0 = b * ROWS_PER_BLK
                        for i in range(KDIM * KDIM):
                            kh, kw = divmod(i, KDIM)
                            w0 = i * COUT + t * 128
                            o = 1 + (r0 + kh) * PWS + (kw - 1)
                            rhs = xf[:, o : o + ROWS_PER_BLK * PWS].rearrange(
                                "p (r c) -> p r c", c=PWS
                            )[:, :, :W]
                            nc.tensor.matmul(
                                ps[:],
                                lhsT=w_sb[:, w0 : w0 + 128],
                                rhs=rhs,
                                start=(i == 0),
                                stop=(i == KDIM * KDIM - 1),
                            )
                        oslice = obig[:, b * BLK_N : (b + 1) * BLK_N]
                        if t == 0:
                            nc.scalar.activation(
                                oslice,
                                ps[:],
                                mybir.ActivationFunctionType.Identity,
                                bias=ctxb[t][:, n : n + 1],
                                scale=1.0,
                            )
                        else:
                            nc.vector.tensor_scalar_add(
                                oslice, ps[:], ctxb[t][:, n : n + 1]
                            )
                    # split the 2MB plane store so the last piece doesn't sit
                    # whole on the kernel's critical tail
                    oflat = out_d[n, t * 128 : (t + 1) * 128].rearrange(
                        "o h w -> o (h w)"
                    )
                    for q in range(4):
                        nc.sync.dma_start(
                            out=oflat[:, q * (HW // 4) : (q + 1) * (HW // 4)],
                            in_=obig[:, q * (HW // 4) : (q + 1) * (HW // 4)],
                        )
                    # prefetch the next image while this one's second
                    # C_out tile computes
                    if t == 0 and n + 1 < IMG:
                        xflats[n + 1] = load_image(n + 1)
    nc.compile()
    return nc


def get_nc():
    global _cached_nc
    if _cached_nc is None:
        _cached_nc = _build()
    return _cached_nc


def prep_in_maps(x, c, weight, c_weight, bias):
    x = np.ascontiguousarray(np.asarray(x, dtype=np.float32))
    c = np.asarray(c, dtype=np.float32)
    weight = np.asarray(weight, dtype=np.float32)
    c_weight = np.asarray(c_weight, dtype=np.float32)
    bias = np.asarray(bias, dtype=np.float32)

    wt = np.ascontiguousarray(
        weight.transpose(2, 3, 1, 0).reshape(KDIM * KDIM, CIN, COUT)
    )
    cwb = np.ascontiguousarray(np.concatenate([c_weight.T, bias[None, :]], axis=0))
    z = np.zeros((CIN, W + 2), np.float32)
    xpad = np.zeros((N_FULL, CIN, H, W + 1), np.float32)
    xpad[:, :, :, :W] = x
    in_maps = []
    for i in range(N_CORES):
        xs = np.ascontiguousarray(xpad[i * IMG : (i + 1) * IMG])
        cb = np.ascontiguousarray(
            np.concatenate(
                [c[i * IMG : (i + 1) * IMG].T, np.ones((1, IMG), np.float32)], axis=0
            )
        )
        in_maps.append({"x": xs, "wt": wt, "cb": cb, "cwb": cwb, "z": z})
    return in_maps


def run(x, c, weight, c_weight, bias, trace=False):
    nc = get_nc()
    in_maps = prep_in_maps(x, c, weight, c_weight, bias)
    res = bass_utils.run_bass_kernel_spmd(
        nc, in_maps, core_ids=list(range(N_CORES)), trace=trace
    )
    out = np.concatenate([res.results[i]["out"] for i in range(N_CORES)], axis=0)
    return out, res


def kernel(x, c, weight, c_weight, bias):
    out, _ = run(x, c, weight, c_weight, bias)
    return out


# revision 26
# speedup vs baseline: 1.0486x; 1.0361x over previous
"""ContextualConv2d Trainium2 kernel.

out = conv2d(x, weight, pad=1) + (c @ c_weight.T)[:, :, None, None] + bias[None, :, None, None]

Full shapes: x (32,128,64,64) f32, c (32,64), weight (256,128,3,3),
c_weight (256,64), bias (256,) -> out (32,256,64,64).

Strategy: data-parallel over batch across 8 NeuronCores (4 images each).
Per core the conv is an implicit GEMM: each image lives in SBUF as
[128ci, 66*64] (two zero rows for the H halo, no W padding so every DMA
is contiguous). For each 128-wide C_out tile and each 512-column output
block (8 image rows x 64 cols) we accumulate 9 matmuls (one per filter
tap) into a PSUM bank using float32r operands (full PE rate at N>=256).
The +-1 column taps use column-sliced matmuls (N=504) with strided PSUM
sub-APs -- the zero-padding contribution is simply never computed; the
full-width center tap runs first in each accumulation group so every
PSUM element is written. The context bias (c @ c_weight.T + bias) comes
from one small on-device matmul per C_out tile (a ones-row on the rhs
folds in the channel bias) and is fused into the PSUM->SBUF epilogue,
which stages a full [128, 4096] image plane for one contiguous 2MB
output DMA per (image, C_out tile).
"""

import numpy as np

import concourse.tile as tile
from concourse import bacc, bass_utils, mybir

N_CORES = 8
N_FULL = 32
IMG = N_FULL // N_CORES  # images per core
CIN = 128
COUT = 256
H = W = 64
HW = H * W
KDIM = 3
CDIM = 64
XROWS = H + 2  # 2 zero rows for the H halo
CO_TILES = COUT // 128
ROWS_PER_BLK = 8
NBLK = H // ROWS_PER_BLK
BLK_N = ROWS_PER_BLK * W  # 512 = one fp32 PSUM bank
F32 = mybir.dt.float32
F32R = mybir.dt.float32r

_cached_nc = None


def _build():
    nc = bacc.Bacc(
        "TRN2",
        target_bir_lowering=False,
        debug=False,
        enable_asserts=False,
        num_devices=N_CORES,
    )
    x_d = nc.dram_tensor("x", (IMG, CIN, H, W + 1), F32R, kind="ExternalInput").ap()
    wt_d = nc.dram_tensor("wt", (KDIM * KDIM, CIN, COUT), F32R, kind="ExternalInput").ap()
    cb_d = nc.dram_tensor("cb", (CDIM + 1, IMG), F32R, kind="ExternalInput").ap()
    cwb_d = nc.dram_tensor("cwb", (CDIM + 1, COUT), F32R, kind="ExternalInput").ap()
    z_d = nc.dram_tensor("z", (CIN, W + 2), F32R, kind="ExternalInput").ap()
    out_d = nc.dram_tensor("out", (IMG, COUT, H, W), F32, kind="ExternalOutput").ap()

    with tile.TileContext(nc) as tc:
        with (
            tc.tile_pool(name="consts", bufs=1) as consts,
            tc.tile_pool(name="xbuf", bufs=1) as xbuf,
            tc.tile_pool(name="obuf", bufs=2) as obuf,
            tc.tile_pool(name="ps", bufs=5, space="PSUM") as pspool,
            tc.tile_pool(name="cps", bufs=1, space="PSUM") as cpspool,
            tc.tile_pool(name="wps", bufs=1, space="PSUM") as wpspool,
        ):
            # PE warmup: the PE idles for the first ~12us waiting on input
            # DMAs, and the HAM clock gate needs ~3.4us of sustained matmul
            # activity to lift the 1.2GHz cold throttle. Run dummy matmuls
            # on a small scratch tile so the real matmuls start at 2.4GHz.
            # The warmup matmuls only exist to keep the PE busy (HAM
            # un-throttle) while the real input DMAs land; their PSUM bank is
            # never read. bf16 zeros: memset is legal for bf16 and the PE
            # rate is the same.
            warm_sb = consts.tile([CIN, BLK_N], mybir.dt.bfloat16)
            nc.gpsimd.memset(warm_sb[:], 0.0)
            wps = wpspool.tile([128, BLK_N], F32)
            for _ in range(10):
                nc.tensor.matmul(
                    wps[:],
                    lhsT=warm_sb[:, 0:128],
                    rhs=warm_sb[:],
                    start=True,
                    stop=True,
                )

            # conv weights lead the scalar-ring FIFO (images follow); the
            # small context tensors and the output stores use the sync ring
            cwb_sb = consts.tile([CDIM + 1, COUT], F32R)
            nc.sync.dma_start(out=cwb_sb[:], in_=cwb_d)
            cb_sb = consts.tile([CDIM + 1, IMG], F32R)
            nc.sync.dma_start(out=cb_sb[:], in_=cb_d)
            w_sb = consts.tile([CIN, KDIM * KDIM * COUT], F32R)
            nc.scalar.dma_start(
                out=w_sb[:].rearrange("p (k o) -> p k o", o=COUT),
                in_=wt_d.transpose([1, 0, 2]),
            )

            # ctxb[t][co, n] = sum_d c_weight[co, d] * c[n, d] + bias[co]
            ctxb = []
            for t in range(CO_TILES):
                cps = cpspool.tile([128, IMG], F32, tag=f"cps{t}")
                nc.tensor.matmul(
                    cps[:],
                    lhsT=cwb_sb[:, t * 128 : (t + 1) * 128],
                    rhs=cb_sb[:],
                    start=True,
                    stop=True,
                )
                csb = consts.tile([128, IMG], F32, tag=f"ctxb{t}")
                nc.vector.tensor_copy(csb[:], cps[:])
                ctxb.append(csb)

            # per-image input planes with stride-65 rows: position
            # 1 + u*PWS + c holds image pixel (u-1, c); column PWS-1 of each
            # row is a zero guard (baked into the host-padded x tensor), and
            # rows 0 / XROWS-1 plus the leading element are zeroed from z_d.
            # The +-1-column taps then read straight through the guards
            # (which contribute zero), so every tap is a uniform N=512
            # matmul with inner-contiguous rhs and a plain 2D PSUM out.
            PWS = W + 1

            def load_image(n):
                """Emit the image-n load: top zero row + leading guard, two
                interior halves, bottom zero row. Fully contiguous DMAs."""
                # one extra row of slack: tap AP slices extend past the last
                # guard before the [:, :, :W] crop trims them
                # image loads ride the scalar (ACT) HWDGE ring, in parallel
                # with the weight load on the sync ring; the first 16 rows
                # land first so block 0 can start early
                xp = xbuf.tile([CIN, 1 + (XROWS + 1) * PWS], F32R, tag=f"ximg{n}")
                xflat = x_d[n].rearrange("p h w -> p (h w)")
                cut = 16 * PWS
                nc.scalar.dma_start(out=xp[:, 0 : 1 + PWS], in_=z_d[:, 0 : 1 + PWS])
                nc.scalar.dma_start(
                    out=xp[:, 1 + PWS : 1 + PWS + cut], in_=xflat[:, 0:cut]
                )
                nc.scalar.dma_start(
                    out=xp[:, 1 + PWS + cut : 1 + (XROWS - 1) * PWS],
                    in_=xflat[:, cut:],
                )
                nc.scalar.dma_start(
                    out=xp[:, 1 + (XROWS - 1) * PWS : 1 + XROWS * PWS],
                    in_=z_d[:, 0:PWS],
                )
                return xp

            xflats = {0: load_image(0)}

            for n in range(IMG):
                xf = xflats[n]
                for t in range(CO_TILES):
                    obig = obuf.tile([128, HW], F32)
                    for b in range(NBLK):
                        ps = pspool.tile([128, BLK_N], F32)
                        r0 = b * ROWS_PER_BLK
                        for i in range(KDIM * KDIM):
                            kh, kw = divmod(i, KDIM)
                            w0 = i * COUT + t * 128
                            o = 1 + (r0 + kh) * PWS + (kw - 1)
                            rhs = xf[:, o : o + ROWS_PER_BLK * PWS].rearrange(
                                "p (r c) -> p r c", c=PWS
                            )[:, :, :W]
                            nc.tensor.matmul(
                                ps[:],
                                lhsT=w_sb[:, w0 : w0 + 128],
                                rhs=rhs,
                                start=(i == 0),
                                stop=(i == KDIM * KDIM - 1),
                            )
                        oslice = obig[:, b * BLK_N : (b + 1) * BLK_N]
                        if t == 0:
                            nc.scalar.activation(
                                oslice,
                                ps[:],
                                mybir.ActivationFunctionType.Identity,
                                bias=ctxb[t][:, n : n + 1],
                                scale=1.0,
                            )
                        else:
                            nc.vector.tensor_scalar_add(
                                oslice, ps[:], ctxb[t][:, n : n + 1]
                            )
                    # split the 2MB plane store so the last piece doesn't sit
                    # whole on the kernel's critical tail
                    oflat = out_d[n, t * 128 : (t + 1) * 128].rearrange(
                        "o h w -> o (h w)"
                    )
                    for q in range(4):
                        nc.sync.dma_start(
                            out=oflat[:, q * (HW // 4) : (q + 1) * (HW // 4)],
                            in_=obig[:, q * (HW // 4) : (q + 1) * (HW // 4)],
                        )
                    # prefetch the next image while this one's second
                    # C_out tile computes
                    if t == 0 and n + 1 < IMG:
                        xflats[n + 1] = load_image(n + 1)
    nc.compile()
    return nc


def get_nc():
    global _cached_nc
    if _cached_nc is None:
        _cached_nc = _build()
    return _cached_nc


def prep_in_maps(x, c, weight, c_weight, bias):
    x = np.ascontiguousarray(np.asarray(x, dtype=np.float32))
    c = np.asarray(c, dtype=np.float32)
    weight = np.asarray(weight, dtype=np.float32)
    c_weight = np.asarray(c_weight, dtype=np.float32)
    bias = np.asarray(bias, dtype=np.float32)

    wt = np.ascontiguousarray(
        weight.transpose(2, 3, 1, 0).reshape(KDIM * KDIM, CIN, COUT)
    )
    cwb = np.ascontiguousarray(np.concatenate([c_weight.T, bias[None, :]], axis=0))
    z = np.zeros((CIN, W + 2), np.float32)
    xpad = np.zeros((N_FULL, CIN, H, W + 1), np.float32)
    xpad[:, :, :, :W] = x
    in_maps = []
    for i in range(N_CORES):
        xs = np.ascontiguousarray(xpad[i * IMG : (i + 1) * IMG])
        cb = np.ascontiguousarray(
            np.concatenate(
                [c[i * IMG : (i + 1) * IMG].T, np.ones((1, IMG), np.float32)], axis=0
            )
        )
        in_maps.append({"x": xs, "wt": wt, "cb": cb, "cwb": cwb, "z": z})
    return in_maps


def run(x, c, weight, c_weight, bias, trace=False):
    nc = get_nc()
    in_maps = prep_in_maps(x, c, weight, c_weight, bias)
    res = bass_utils.run_bass_kernel_spmd(
        nc, in_maps, core_ids=list(range(N_CORES)), trace=trace
    )
    out = np.concatenate([res.results[i]["out"] for i in range(N_CORES)], axis=0)
    return out, res


def kernel(x, c, weight, c_weight, bias):
    out, _ = run(x, c, weight, c_weight, bias)
    return out
